# revision 10
# baseline (speedup 1.0000x reference)
"""minGRU Trainium2 Bass kernel.

Reference computation (per batch b):
    hidden = x @ W_hidden            [S, Di]
    gate   = x @ W_gate              [S, Di]
    a_t    = sigmoid(-gate)          (= exp(log_coeffs) = 1 - z)
    z_t    = sigmoid(gate)
    g(h)   = h + 0.5 if h >= 0 else sigmoid(h)
    b_t    = z_t * g(hidden_t)
    h_t    = a_t * h_{t-1} + b_t     (h_{-1} = 0; linear-space scan,
                                      numerically stable: convex combination)
    out    = h @ W_out               [S, D]

Sharding over 8 cores: (batch b in 0..3) x (half of Di). Each core computes
its batch's projections against its 768-column slice of W_hidden/W_gate,
scans, and multiplies by its 768-row slice of W_out, producing a partial
[D, S] (transposed) output. Host adds the two halves and transposes back.

Device layout: everything keeps the sequence on the free axis and
features/d-model on partitions, so no on-device transposes are needed:
    x is fed pre-transposed as xT [D, S];
    proj matmul: out[f, s] = sum_d Wh[d, f] * xT[d, s]  (lhsT = Wh, rhs = xT)
    scan: tensor_tensor_scan along the free (sequence) axis
    out matmul: outT[d, s] = sum_f Wo[f, d] * h[f, s]   (lhsT = Wo, rhs = h)
Matmuls run in float32r (full fp32 data, reduced-precision PE multiply,
1 cycle/row vs 4 for strict fp32).
"""

import numpy as np
from contextlib import ExitStack

import concourse.bass as bass
import concourse.tile as tile
from concourse import bacc, mybir
from concourse.bass_utils import run_bass_kernel_spmd

B = 4
S = 4096
D = 1024
DI = 1536
F = DI // 2            # 768 features per core
N_CORES = 8
SC = 512               # sequence chunk (one PSUM bank of fp32)
KD = D // 128          # 8 contraction tiles for the projections
NF = F // 128          # 6 feature tiles per core
ND = D // 128          # 8 output d-model tiles

F32 = mybir.dt.float32
F32R = mybir.dt.float32r
ACT = mybir.ActivationFunctionType
ALU = mybir.AluOpType

_cache = {}


def _build(seq_len=S, matmul_dtype=F32R, reps=1, timing=False):
    nsc = seq_len // SC
    nc = bacc.Bacc("TRN2", target_bir_lowering=False, debug=False,
                   num_devices=N_CORES)
    md = matmul_dtype
    if timing:
        # Timing build: all big tensors stay device-internal (their values
        # are irrelevant for speed) so repeated calls ship only a token
        # through the axon tunnel.
        xT = nc.dram_tensor("xT", [D, seq_len], md).ap()
        wh = nc.dram_tensor("wh", [D, F], md).ap()
        wg = nc.dram_tensor("wg", [D, F], md).ap()
        wo = nc.dram_tensor("wo", [F, D], md).ap()
        outT = nc.dram_tensor("outT", [D, seq_len], F32).ap()
        seed = nc.dram_tensor("seed", [1, 8], F32, kind="ExternalInput").ap()
        done = nc.dram_tensor("done", [1, 8 * reps], F32,
                              kind="ExternalOutput").ap()
    else:
        xT = nc.dram_tensor("xT", [D, seq_len], md, kind="ExternalInput").ap()
        wh = nc.dram_tensor("wh", [D, F], md, kind="ExternalInput").ap()
        wg = nc.dram_tensor("wg", [D, F], md, kind="ExternalInput").ap()
        wo = nc.dram_tensor("wo", [F, D], md, kind="ExternalInput").ap()
        outT = nc.dram_tensor("outT", [D, seq_len], F32,
                              kind="ExternalOutput").ap()
        seed = None
        done = None

    with tile.TileContext(nc) as tc, ExitStack() as ctx:
        wpool = ctx.enter_context(tc.tile_pool(name="w", bufs=1))
        xpool = ctx.enter_context(tc.tile_pool(name="x", bufs=2))
        ppool = ctx.enter_context(tc.tile_pool(name="pp", bufs=2, space="PSUM"))
        opool = ctx.enter_context(tc.tile_pool(name="po", bufs=3, space="PSUM"))
        epool = ctx.enter_context(tc.tile_pool(name="e", bufs=2))
        hpool = ctx.enter_context(tc.tile_pool(name="h", bufs=2))
        spool = ctx.enter_context(tc.tile_pool(name="os", bufs=3))

        # Resident weights. Column block dk of wh_sb/wg_sb holds rows
        # dk*128..+128 of the [D, F] weight; block fk of wo_sb holds rows
        # fk*128..+128 of the [F, D] weight.
        wh_sb = wpool.tile([128, KD * F], md, tag="wh")
        wg_sb = wpool.tile([128, KD * F], md, tag="wg")
        wo_sb = wpool.tile([128, NF * D], md, tag="wo")
        for dk in range(KD):
            nc.sync.dma_start(wh_sb[:, dk * F:(dk + 1) * F],
                              wh[dk * 128:(dk + 1) * 128, :])
            nc.sync.dma_start(wg_sb[:, dk * F:(dk + 1) * F],
                              wg[dk * 128:(dk + 1) * 128, :])
        for fk in range(NF):
            nc.sync.dma_start(wo_sb[:, fk * D:(fk + 1) * D],
                              wo[fk * 128:(fk + 1) * 128, :])

        for _rep in range(reps):
          h_prev = [None] * NF
          for sc in range(nsc):
            x_sb = xpool.tile([128, KD * SC], md, tag="x")
            for dk in range(KD):
                nc.sync.dma_start(
                    x_sb[:, dk * SC:(dk + 1) * SC],
                    xT[dk * 128:(dk + 1) * 128, sc * SC:(sc + 1) * SC])

            h_cur = []
            for ft in range(NF):
                ph = ppool.tile([128, SC], F32, tag="ph")
                pg = ppool.tile([128, SC], F32, tag="pg")
                for dk in range(KD):
                    cw = dk * F + ft * 128
                    rx = x_sb[:, dk * SC:(dk + 1) * SC]
                    nc.tensor.matmul(
                        ph[:], wh_sb[:, cw:cw + 128],
                        rx, start=(dk == 0), stop=(dk == KD - 1))
                    nc.tensor.matmul(
                        pg[:], wg_sb[:, cw:cw + 128],
                        rx, start=(dk == 0), stop=(dk == KD - 1))

                z_sb = epool.tile([128, SC], F32, tag="z")
                a_sb = epool.tile([128, SC], F32, tag="a")
                s_sb = epool.tile([128, SC], F32, tag="s")
                r_sb = epool.tile([128, SC], F32, tag="r")
                g_sb = epool.tile([128, SC], F32, tag="g")
                b_sb = epool.tile([128, SC], F32, tag="b")
                nc.scalar.activation(z_sb[:], pg[:], ACT.Sigmoid)
                nc.scalar.activation(a_sb[:], pg[:], ACT.Sigmoid, scale=-1.0)
                nc.scalar.activation(s_sb[:], ph[:], ACT.Sigmoid)
                nc.scalar.activation(r_sb[:], ph[:], ACT.Relu)
                # g = min(sigmoid(h), 0.5) + relu(h)
                nc.vector.scalar_tensor_tensor(
                    g_sb[:], s_sb[:], 0.5, r_sb[:], op0=ALU.min, op1=ALU.add)
                nc.vector.tensor_mul(b_sb[:], z_sb[:], g_sb[:])

                h_sb = hpool.tile([128, SC], md, tag=f"h{ft}")
                init = 0.0 if sc == 0 else h_prev[ft][:, SC - 1:SC]
                nc.vector.tensor_tensor_scan(
                    h_sb[:], a_sb[:], b_sb[:], init,
                    op0=ALU.mult, op1=ALU.add)
                h_cur.append(h_sb)

            for dt_ in range(ND):
                po = opool.tile([128, SC], F32, tag="po")
                for fk in range(NF):
                    cw = fk * D + dt_ * 128
                    nc.tensor.matmul(
                        po[:], wo_sb[:, cw:cw + 128],
                        h_cur[fk][:],
                        start=(fk == 0), stop=(fk == NF - 1))
                o_sb = spool.tile([128, SC], F32, tag="o")
                # PSUM has no DMA route; alternate the copy between engines.
                if dt_ % 2 == 0:
                    nc.scalar.copy(o_sb[:], po[:])
                else:
                    nc.vector.tensor_copy(o_sb[:], po[:])
                nc.sync.dma_start(
                    outT[dt_ * 128:(dt_ + 1) * 128, sc * SC:(sc + 1) * SC],
                    o_sb[:])
            h_prev = h_cur

          if timing and _rep == reps - 1:
            tok = spool.tile([1, 8 * reps], F32, tag="tok")
            nc.vector.memset(tok[:], 1.0)
            nc.sync.dma_start(done[:], tok[:])

    nc.compile()
    return nc


def get_nc(seq_len=S, matmul_dtype=F32R, reps=1, timing=False):
    key = (seq_len, matmul_dtype, reps, timing)
    if key not in _cache:
        _cache[key] = _build(seq_len, matmul_dtype, reps, timing)
    return _cache[key]


def make_in_maps(x, W_hidden, W_gate, W_out, matmul_dtype=F32R):
    """Shard full inputs into per-core input maps (core c -> batch c//2,
    Di-half c%2)."""
    np_dt = mybir.dt.np(matmul_dtype)
    in_maps = []
    xT = np.ascontiguousarray(np.transpose(x, (0, 2, 1))).astype(np_dt)
    for c in range(N_CORES):
        b, hf = divmod(c, 2)
        in_maps.append({
            "xT": xT[b],
            "wh": np.ascontiguousarray(W_hidden[:, hf * F:(hf + 1) * F]).astype(np_dt),
            "wg": np.ascontiguousarray(W_gate[:, hf * F:(hf + 1) * F]).astype(np_dt),
            "wo": np.ascontiguousarray(W_out[hf * F:(hf + 1) * F, :]).astype(np_dt),
        })
    return in_maps


def assemble(results):
    """Combine per-core partial transposed outputs into [B, S, D]."""
    out = np.empty((B, S, D), np.float32)
    for b in range(B):
        acc = results[2 * b]["outT"] + results[2 * b + 1]["outT"]  # [D, S]
        out[b] = acc.T
    return out


def kernel(x, W_hidden, W_gate, W_out):
    x = np.asarray(x, np.float32)
    W_hidden = np.asarray(W_hidden, np.float32)
    W_gate = np.asarray(W_gate, np.float32)
    W_out = np.asarray(W_out, np.float32)
    nc = get_nc()
    in_maps = make_in_maps(x, W_hidden, W_gate, W_out)
    last_err = None
    for attempt in range(3):
        try:
            res = run_bass_kernel_spmd(nc, in_maps, list(range(N_CORES)))
            return assemble(res.results)
        except Exception as e:  # transient device faults under axon
            last_err = e
            import time as _time
            _time.sleep(5.0 * (attempt + 1))
    raise last_err


# revision 11
# speedup vs baseline: 1.0719x; 1.0719x over previous
"""minGRU Trainium2 Bass kernel.

Reference computation (per batch b):
    hidden = x @ W_hidden            [S, Di]
    gate   = x @ W_gate              [S, Di]
    a_t    = sigmoid(-gate)          (= exp(log_coeffs) = 1 - z)
    z_t    = sigmoid(gate)
    g(h)   = h + 0.5 if h >= 0 else sigmoid(h)
    b_t    = z_t * g(hidden_t)
    h_t    = a_t * h_{t-1} + b_t     (h_{-1} = 0; linear-space scan,
                                      numerically stable: convex combination)
    out    = h @ W_out               [S, D]

Sharding over 8 cores: (batch b in 0..3) x (half of Di). Each core computes
its batch's projections against its 768-column slice of W_hidden/W_gate,
scans, and multiplies by its 768-row slice of W_out, producing a partial
[D, S] (transposed) output. Host adds the two halves and transposes back.

Device layout: everything keeps the sequence on the free axis and
features/d-model on partitions, so no on-device transposes are needed:
    x is fed pre-transposed as xT [D, S];
    proj matmul: out[f, s] = sum_d Wh[d, f] * xT[d, s]  (lhsT = Wh, rhs = xT)
    scan: tensor_tensor_scan along the free (sequence) axis
    out matmul: outT[d, s] = sum_f Wo[f, d] * h[f, s]   (lhsT = Wo, rhs = h)
Matmuls run in float32r (full fp32 data, reduced-precision PE multiply,
1 cycle/row vs 4 for strict fp32).
"""

import numpy as np
from contextlib import ExitStack

import concourse.bass as bass
import concourse.tile as tile
from concourse import bacc, mybir
from concourse.bass_utils import run_bass_kernel_spmd

B = 4
S = 4096
D = 1024
DI = 1536
F = DI // 2            # 768 features per core
N_CORES = 8
SC = 512               # sequence chunk (one PSUM bank of fp32)
KD = D // 128          # 8 contraction tiles for the projections
NF = F // 128          # 6 feature tiles per core
ND = D // 128          # 8 output d-model tiles

F32 = mybir.dt.float32
F32R = mybir.dt.float32r
ACT = mybir.ActivationFunctionType
ALU = mybir.AluOpType

_cache = {}


def _build(seq_len=S, matmul_dtype=F32R, reps=1, timing=False):
    nsc = seq_len // SC
    nc = bacc.Bacc("TRN2", target_bir_lowering=False, debug=False,
                   num_devices=N_CORES)
    md = matmul_dtype
    if timing:
        # Timing build: all big tensors stay device-internal (their values
        # are irrelevant for speed) so repeated calls ship only a token
        # through the axon tunnel.
        xT = nc.dram_tensor("xT", [D, seq_len], md).ap()
        wh = nc.dram_tensor("wh", [D, F], md).ap()
        wg = nc.dram_tensor("wg", [D, F], md).ap()
        wo = nc.dram_tensor("wo", [F, D], md).ap()
        outT = nc.dram_tensor("outT", [D, seq_len], F32).ap()
        seed = nc.dram_tensor("seed", [1, 8], F32, kind="ExternalInput").ap()
        done = nc.dram_tensor("done", [1, 8 * reps], F32,
                              kind="ExternalOutput").ap()
    else:
        xT = nc.dram_tensor("xT", [D, seq_len], md, kind="ExternalInput").ap()
        wh = nc.dram_tensor("wh", [D, F], md, kind="ExternalInput").ap()
        wg = nc.dram_tensor("wg", [D, F], md, kind="ExternalInput").ap()
        wo = nc.dram_tensor("wo", [F, D], md, kind="ExternalInput").ap()
        outT = nc.dram_tensor("outT", [D, seq_len], F32,
                              kind="ExternalOutput").ap()
        seed = None
        done = None

    with tile.TileContext(nc) as tc, ExitStack() as ctx:
        wpool = ctx.enter_context(tc.tile_pool(name="w", bufs=1))
        xpool = ctx.enter_context(tc.tile_pool(name="x", bufs=3))
        ppool = ctx.enter_context(tc.tile_pool(name="pp", bufs=2, space="PSUM"))
        opool = ctx.enter_context(tc.tile_pool(name="po", bufs=4, space="PSUM"))
        epool = ctx.enter_context(tc.tile_pool(name="e", bufs=2))
        hpool = ctx.enter_context(tc.tile_pool(name="h", bufs=2))
        spool = ctx.enter_context(tc.tile_pool(name="os", bufs=3))

        # Resident weights. Column block dk of wh_sb/wg_sb holds rows
        # dk*128..+128 of the [D, F] weight; block fk of wo_sb holds rows
        # fk*128..+128 of the [F, D] weight.
        wh_sb = wpool.tile([128, KD * F], md, tag="wh")
        wg_sb = wpool.tile([128, KD * F], md, tag="wg")
        wo_sb = wpool.tile([128, NF * D], md, tag="wo")
        for dk in range(KD):
            nc.sync.dma_start(wh_sb[:, dk * F:(dk + 1) * F],
                              wh[dk * 128:(dk + 1) * 128, :])
            nc.sync.dma_start(wg_sb[:, dk * F:(dk + 1) * F],
                              wg[dk * 128:(dk + 1) * 128, :])
        for fk in range(NF):
            nc.sync.dma_start(wo_sb[:, fk * D:(fk + 1) * D],
                              wo[fk * 128:(fk + 1) * 128, :])

        for _rep in range(reps):
          h_prev = [None] * NF
          for sc in range(nsc):
            x_sb = xpool.tile([128, KD * SC], md, tag="x")
            for dk in range(KD):
                nc.sync.dma_start(
                    x_sb[:, dk * SC:(dk + 1) * SC],
                    xT[dk * 128:(dk + 1) * 128, sc * SC:(sc + 1) * SC])

            h_cur = []
            for ft in range(NF):
                ph = ppool.tile([128, SC], F32, tag="ph")
                pg = ppool.tile([128, SC], F32, tag="pg")
                for dk in range(KD):
                    cw = dk * F + ft * 128
                    rx = x_sb[:, dk * SC:(dk + 1) * SC]
                    nc.tensor.matmul(
                        ph[:], wh_sb[:, cw:cw + 128],
                        rx, start=(dk == 0), stop=(dk == KD - 1))
                    nc.tensor.matmul(
                        pg[:], wg_sb[:, cw:cw + 128],
                        rx, start=(dk == 0), stop=(dk == KD - 1))

                z_sb = epool.tile([128, SC], F32, tag="z")
                a_sb = epool.tile([128, SC], F32, tag="a")
                s_sb = epool.tile([128, SC], F32, tag="s")
                r_sb = epool.tile([128, SC], F32, tag="r")
                g_sb = epool.tile([128, SC], F32, tag="g")
                b_sb = epool.tile([128, SC], F32, tag="b")
                nc.scalar.activation(z_sb[:], pg[:], ACT.Sigmoid)
                nc.scalar.activation(a_sb[:], pg[:], ACT.Sigmoid, scale=-1.0)
                nc.scalar.activation(s_sb[:], ph[:], ACT.Sigmoid)
                nc.scalar.activation(r_sb[:], ph[:], ACT.Relu)
                # g = min(sigmoid(h), 0.5) + relu(h)
                nc.vector.scalar_tensor_tensor(
                    g_sb[:], s_sb[:], 0.5, r_sb[:], op0=ALU.min, op1=ALU.add)
                nc.vector.tensor_mul(b_sb[:], z_sb[:], g_sb[:])

                h_sb = hpool.tile([128, SC], md, tag=f"h{ft}")
                init = 0.0 if sc == 0 else h_prev[ft][:, SC - 1:SC]
                nc.vector.tensor_tensor_scan(
                    h_sb[:], a_sb[:], b_sb[:], init,
                    op0=ALU.mult, op1=ALU.add)
                h_cur.append(h_sb)

            for dt_ in range(ND):
                po = opool.tile([128, SC], F32, tag="po")
                for fk in range(NF):
                    cw = fk * D + dt_ * 128
                    nc.tensor.matmul(
                        po[:], wo_sb[:, cw:cw + 128],
                        h_cur[fk][:],
                        start=(fk == 0), stop=(fk == NF - 1))
                o_sb = spool.tile([128, SC], F32, tag="o")
                # PSUM has no DMA route; alternate the copy between engines.
                if dt_ % 2 == 0:
                    nc.scalar.copy(o_sb[:], po[:])
                else:
                    nc.vector.tensor_copy(o_sb[:], po[:])
                nc.sync.dma_start(
                    outT[dt_ * 128:(dt_ + 1) * 128, sc * SC:(sc + 1) * SC],
                    o_sb[:])
            h_prev = h_cur

          if timing and _rep == reps - 1:
            tok = spool.tile([1, 8 * reps], F32, tag="tok")
            nc.vector.memset(tok[:], 1.0)
            nc.sync.dma_start(done[:], tok[:])

    nc.compile()
    return nc


def get_nc(seq_len=S, matmul_dtype=F32R, reps=1, timing=False):
    key = (seq_len, matmul_dtype, reps, timing)
    if key not in _cache:
        _cache[key] = _build(seq_len, matmul_dtype, reps, timing)
    return _cache[key]


def make_in_maps(x, W_hidden, W_gate, W_out, matmul_dtype=F32R):
    """Shard full inputs into per-core input maps (core c -> batch c//2,
    Di-half c%2)."""
    np_dt = mybir.dt.np(matmul_dtype)
    in_maps = []
    xT = np.ascontiguousarray(np.transpose(x, (0, 2, 1))).astype(np_dt)
    for c in range(N_CORES):
        b, hf = divmod(c, 2)
        in_maps.append({
            "xT": xT[b],
            "wh": np.ascontiguousarray(W_hidden[:, hf * F:(hf + 1) * F]).astype(np_dt),
            "wg": np.ascontiguousarray(W_gate[:, hf * F:(hf + 1) * F]).astype(np_dt),
            "wo": np.ascontiguousarray(W_out[hf * F:(hf + 1) * F, :]).astype(np_dt),
        })
    return in_maps


def assemble(results):
    """Combine per-core partial transposed outputs into [B, S, D]."""
    out = np.empty((B, S, D), np.float32)
    for b in range(B):
        acc = results[2 * b]["outT"] + results[2 * b + 1]["outT"]  # [D, S]
        out[b] = acc.T
    return out


def kernel(x, W_hidden, W_gate, W_out):
    x = np.asarray(x, np.float32)
    W_hidden = np.asarray(W_hidden, np.float32)
    W_gate = np.asarray(W_gate, np.float32)
    W_out = np.asarray(W_out, np.float32)
    nc = get_nc()
    in_maps = make_in_maps(x, W_hidden, W_gate, W_out)
    last_err = None
    for attempt in range(3):
        try:
            res = run_bass_kernel_spmd(nc, in_maps, list(range(N_CORES)))
            return assemble(res.results)
        except Exception as e:  # transient device faults under axon
            last_err = e
            import time as _time
            _time.sleep(5.0 * (attempt + 1))
    raise last_err
